# revision 6
# baseline (speedup 1.0000x reference)
"""Bass/Trainium2 kernel for nn_DWAMiddleLayer (low-rank MoE weight-assembly layer).

Math (reference):
    U    = pool[:, :1024].reshape(N, DB, R)      # [512, 256, 4]
    V    = pool[:, 1024:2048].reshape(N, R, DA)  # [512, 4, 256]
    bE   = pool[:, 2048:2304]                    # [512, 256]
    h_t  = h_A @ W_base.T
           + sum_r (alpha * (h_A @ V_r.T)) @ U_r          # never materialize W_assembled
           + alpha @ bE + b_base
    y    = h_A + gamma * h_t ; out = LayerNorm(y) * ln_scale + ln_bias

Distribution: data-parallel over batch B=2048 across 8 cores (BS=256 rows each);
pool/W_base/vectors replicated. h_t is computed in transposed space (feature dim
on partitions, batch on the free dim) so that every matmul contraction dim lands
on partitions naturally; transposes of h_A / alpha / V are done on the PE as
bf16 identity-matmuls. All matmul operands are bf16 (the gamma=1e-2 residual
scaling makes matmul rounding error negligible in the output); the residual +
LayerNorm path uses the untransposed fp32 h_A directly.
"""

import numpy as np

B, N, D_A, D_B, R = 2048, 512, 256, 256, 4
NC_COUNT = 8
BS = B // NC_COUNT  # 256 batch rows per core
P = 128
LN_EPS = 1e-5

_cache = {}


def _build_nc():
    import concourse.mybir as mybir
    from concourse import bacc
    import concourse.tile as tile
    from concourse.masks import make_identity

    fp32 = mybir.dt.float32
    bf16 = mybir.dt.bfloat16

    nc = bacc.Bacc("TRN2", target_bir_lowering=False)

    # ---- DRAM I/O (per-core shard shapes) ----
    d_hA = nc.dram_tensor("h_A", [BS, D_A], fp32, kind="ExternalInput")
    d_alpha = nc.dram_tensor("alpha", [BS, N], fp32, kind="ExternalInput")
    d_U = nc.dram_tensor("Upool", [N, D_B * R], fp32, kind="ExternalInput")
    d_V = nc.dram_tensor("Vpool", [N, R * D_A], fp32, kind="ExternalInput")
    d_bE = nc.dram_tensor("biasE", [N, D_B], fp32, kind="ExternalInput")
    d_Wb = nc.dram_tensor("W_base", [D_B, D_A], fp32, kind="ExternalInput")
    d_bb = nc.dram_tensor("b_base", [D_B], fp32, kind="ExternalInput")
    d_gamma = nc.dram_tensor("gamma", [1, 1], fp32, kind="ExternalInput")
    d_lsc = nc.dram_tensor("ln_scale", [D_A], fp32, kind="ExternalInput")
    d_lbi = nc.dram_tensor("ln_bias", [D_A], fp32, kind="ExternalInput")
    d_out = nc.dram_tensor("out", [BS, D_A], fp32, kind="ExternalOutput")

    with tile.TileContext(nc) as tc:
        with (
            tc.tile_pool(name="persist", bufs=1) as persist,
            tc.tile_pool(name="stage", bufs=2) as stage,
            tc.tile_pool(name="sm", bufs=3) as sm,
            tc.tile_pool(name="pp_tr", bufs=2, space="PSUM") as pp_tr,
            tc.tile_pool(name="pp_t", bufs=2, space="PSUM") as pp_t,
            tc.tile_pool(name="pp_acc", bufs=1, space="PSUM") as pp_acc,
            tc.tile_pool(name="pp_y", bufs=1, space="PSUM") as pp_y,
        ):
            # ---------- small loads ----------
            hA_sb = persist.tile([P, 2, D_A], fp32)  # [p, b_chunk, a]
            nc.sync.dma_start(hA_sb, d_hA[:].rearrange("(o p) a -> p o a", p=P))
            alpha_sb = persist.tile([P, 2, N], fp32)
            nc.sync.dma_start(alpha_sb, d_alpha[:].rearrange("(o p) n -> p o n", p=P))
            Wb_sb = persist.tile([P, 2, D_A], fp32)
            nc.sync.dma_start(Wb_sb, d_Wb[:].rearrange("(o p) a -> p o a", p=P))
            bE_sb = persist.tile([P, 4, D_B], fp32)
            nc.sync.dma_start(bE_sb, d_bE[:].rearrange("(o p) c -> p o c", p=P))

            # broadcast rows / misc constants
            lsc_row = persist.tile([P, D_A], fp32)
            nc.gpsimd.dma_start(lsc_row, d_lsc[:].partition_broadcast(P))
            lbi_row = persist.tile([P, D_A], fp32)
            nc.gpsimd.dma_start(lbi_row, d_lbi[:].partition_broadcast(P))
            gamma_col = persist.tile([P, 1], fp32)
            nc.gpsimd.dma_start(gamma_col, d_gamma[:].to_broadcast([P, 1]))
            bb_row = persist.tile([1, D_B], fp32)  # b_base on one partition
            nc.gpsimd.dma_start(bb_row, d_bb[:].unsqueeze(0))

            eps_col = persist.tile([P, 1], fp32)
            nc.vector.memset(eps_col, LN_EPS)
            ident_b = persist.tile([P, P], bf16)
            make_identity(nc, ident_b)
            ones_row = persist.tile([1, BS], bf16)
            nc.vector.memset(ones_row, 1.0)
            bb_bf = persist.tile([1, D_B], bf16)
            nc.vector.tensor_copy(bb_bf, bb_row)

            # ---------- transposes of small operands (PE identity-matmul, bf16) ----------
            # h_A^T (bf16, matmul use only; the fp32 residual uses hA_sb directly)
            hA_bf = sm.tile([P, 2, D_A], bf16, tag="hAbf")
            nc.vector.tensor_copy(hA_bf, hA_sb)
            hAT_b = persist.tile([P, 2, BS], bf16)  # [p_a, a_chunk, b]
            for ach in range(2):
                ps = pp_tr.tile([P, 512], fp32, tag="tr")
                for bch in range(2):
                    nc.tensor.matmul(
                        ps[:, bch * P : (bch + 1) * P],
                        lhsT=hA_bf[:, bch, ach * P : (ach + 1) * P],
                        rhs=ident_b,
                        start=True,
                        stop=True,
                    )
                nc.any.tensor_copy(hAT_b[:, ach], ps[:, :BS])

            # alpha^T -> bf16 [p_n, n_chunk, b]
            alpha_bf = persist.tile([P, 2, N], bf16)
            nc.gpsimd.tensor_copy(alpha_bf, alpha_sb)
            alphaT_b = persist.tile([P, 4, BS], bf16)
            for och in range(4):
                ps = pp_tr.tile([P, 512], fp32, tag="tr")
                for bch in range(2):
                    nc.tensor.matmul(
                        ps[:, bch * P : (bch + 1) * P],
                        lhsT=alpha_bf[:, bch, och * P : (och + 1) * P],
                        rhs=ident_b,
                        start=True,
                        stop=True,
                    )
                nc.any.tensor_copy(alphaT_b[:, och], ps[:, :BS])

            # W_base^T -> bf16 [p_a, a_chunk, c]
            Wb_bf = sm.tile([P, 2, D_A], bf16, tag="Wbbf")
            nc.gpsimd.tensor_copy(Wb_bf, Wb_sb)
            WbT_b = persist.tile([P, 2, D_B], bf16)
            for ach in range(2):
                ps = pp_tr.tile([P, 512], fp32, tag="tr")
                for cch in range(2):
                    nc.tensor.matmul(
                        ps[:, cch * P : (cch + 1) * P],
                        lhsT=Wb_bf[:, cch, ach * P : (ach + 1) * P],
                        rhs=ident_b,
                        start=True,
                        stop=True,
                    )
                nc.any.tensor_copy(WbT_b[:, ach], ps[:, :D_B])

            # biasE -> bf16
            bE_bf = persist.tile([P, 4, D_B], bf16)
            nc.gpsimd.tensor_copy(bE_bf, bE_sb)

            # ---------- h_t^T accumulator: 2 psum tiles [c_half, b] ----------
            htT = [
                pp_acc.tile([P, BS], fp32, tag=f"acc{ch}", name=f"htT{ch}")
                for ch in range(2)
            ]

            # base-mm: h_t^T += W_base^T-chunks @ h_A^T   (contraction over a)
            for ch in range(2):
                for ach in range(2):
                    nc.tensor.matmul(
                        htT[ch],
                        lhsT=WbT_b[:, ach, ch * P : (ch + 1) * P],
                        rhs=hAT_b[:, ach],
                        start=(ach == 0),
                        stop=False,
                        skip_group_check=True,
                    )
            # b_base: rank-1 update b_base[c] * ones[b]
            for ch in range(2):
                nc.tensor.matmul(
                    htT[ch],
                    lhsT=bb_bf[:, ch * P : (ch + 1) * P],
                    rhs=ones_row,
                    start=False,
                    stop=False,
                    skip_group_check=True,
                )
            # bias-mm: h_t^T += biasE^T-chunks @ alpha^T  (contraction over n)
            for ch in range(2):
                for och in range(4):
                    nc.tensor.matmul(
                        htT[ch],
                        lhsT=bE_bf[:, och, ch * P : (ch + 1) * P],
                        rhs=alphaT_b[:, och],
                        start=False,
                        stop=False,
                        skip_group_check=True,
                    )

            # ---------- main pipeline over expert chunks (o = n//128) ----------
            # V layout per pool row: f = r*256 + a  (r-major)
            # U layout per pool row: f = c*4 + r    (c-major)
            VT_b = persist.tile([P, 2, 2048], bf16)  # [p_a, a_chunk, r*512+o*128+pn]
            U_bfr = persist.tile([P, 4, R, D_B], bf16)  # [p_n, o, r, c]

            for o in range(4):
                V_st = stage.tile([P, R * D_A], fp32, tag="vst")
                nc.sync.dma_start(V_st, d_V[o * P : (o + 1) * P, :])
                V_bf = stage.tile([P, R * D_A], bf16, tag="vbf")
                nc.gpsimd.tensor_copy(V_bf, V_st)

                # transpose V chunk: blocks (r, a_half) of [128n x 128a]
                for ach in range(2):
                    ps = pp_tr.tile([P, 512], fp32, tag="tr")
                    for r in range(4):
                        nc.tensor.matmul(
                            ps[:, r * P : (r + 1) * P],
                            lhsT=V_bf[:, r * D_A + ach * P : r * D_A + (ach + 1) * P],
                            rhs=ident_b,
                            start=True,
                            stop=True,
                        )
                    # scatter the 4 r-blocks into VT at [r*512 + o*128]
                    dst = VT_b[:, ach].rearrange("p (r q) -> p r q", r=4)[
                        :, :, o * P : (o + 1) * P
                    ]
                    nc.any.tensor_copy(dst, ps[:].rearrange("p (r q) -> p r q", r=4))

                U_st = stage.tile([P, D_B * R], fp32, tag="ust")
                nc.sync.dma_start(U_st, d_U[o * P : (o + 1) * P, :])
                # destride (c r) -> (r c) while casting
                nc.gpsimd.tensor_copy(
                    U_bfr[:, o],
                    U_st.rearrange("p (c r) -> p r c", r=R),
                )

                for r in range(4):
                    # mm1: t_r^T[n_chunk, b] = V_r @ h_A^T (contract a)
                    t_ps = pp_t.tile([P, BS], fp32, tag="t")
                    for ach in range(2):
                        nc.tensor.matmul(
                            t_ps,
                            lhsT=VT_b[:, ach, r * 512 + o * P : r * 512 + (o + 1) * P],
                            rhs=hAT_b[:, ach],
                            start=(ach == 0),
                            stop=(ach == 1),
                        )
                    # s_r^T = alpha^T * t_r^T  (evict psum -> bf16 sbuf)
                    s_bf = sm.tile([P, BS], bf16, tag="s")
                    nc.vector.tensor_mul(s_bf, t_ps, alphaT_b[:, o])
                    # mm2: h_t^T += U_r^T-chunks @ s_r^T (contract n)
                    for ch in range(2):
                        nc.tensor.matmul(
                            htT[ch],
                            lhsT=U_bfr[:, o, r, ch * P : (ch + 1) * P],
                            rhs=s_bf,
                            start=False,
                            stop=(o == 3 and r == 3),
                            skip_group_check=True,
                        )

            # ---------- epilogue: transpose h_t back, residual + LayerNorm in fp32 ----------
            htT_bf = sm.tile([P, 2, BS], bf16, tag="htTbf")
            for ch in range(2):
                nc.any.tensor_copy(htT_bf[:, ch], htT[ch])

            ht_ps = [
                pp_y.tile([P, D_A], fp32, tag=f"y{b}", name=f"ht_ps{b}")
                for b in range(2)
            ]
            for bch in range(2):
                for jch in range(2):
                    nc.tensor.matmul(
                        ht_ps[bch][:, jch * P : (jch + 1) * P],
                        lhsT=htT_bf[:, jch, bch * P : (bch + 1) * P],
                        rhs=ident_b,
                        start=True,
                        stop=True,
                        skip_group_check=True,
                    )

            out_sb = sm.tile([P, 2, D_A], fp32, tag="out")
            for bch in range(2):
                # y = h_A + gamma * h_t   (fp32 residual)
                y_sb = sm.tile([P, D_A], fp32, tag="y")
                nc.vector.scalar_tensor_tensor(
                    y_sb,
                    in0=ht_ps[bch],
                    scalar=gamma_col,
                    in1=hA_sb[:, bch],
                    op0=mybir.AluOpType.mult,
                    op1=mybir.AluOpType.add,
                )
                stats = sm.tile([P, 6], fp32, tag="st")
                nc.vector.bn_stats(stats, y_sb)
                mv = sm.tile([P, 2], fp32, tag="mv")
                nc.vector.bn_aggr(mv, stats)
                # rstd = 1/sqrt(var + eps)
                rstd = sm.tile([P, 1], fp32, tag="rstd")
                nc.scalar.activation(
                    rstd, mv[:, 1:2], mybir.ActivationFunctionType.Sqrt, bias=eps_col
                )
                nc.vector.reciprocal(rstd, rstd)
                # (y - mu) * rstd
                nc.vector.tensor_scalar(
                    out_sb[:, bch],
                    y_sb,
                    scalar1=mv[:, 0:1],
                    scalar2=rstd,
                    op0=mybir.AluOpType.subtract,
                    op1=mybir.AluOpType.mult,
                )
                # * ln_scale + ln_bias
                nc.vector.tensor_mul(out_sb[:, bch], out_sb[:, bch], lsc_row)
                nc.vector.tensor_add(out_sb[:, bch], out_sb[:, bch], lbi_row)

            nc.sync.dma_start(d_out[:].rearrange("(o p) c -> p o c", p=P), out_sb)

    nc.compile()
    return nc


def _get_nc():
    if "nc" not in _cache:
        _cache["nc"] = _build_nc()
    return _cache["nc"]


def make_in_maps(**inputs):
    """Shard full inputs into 8 per-core input maps."""
    f32 = lambda x: np.ascontiguousarray(np.asarray(x), dtype=np.float32)
    h_A = f32(inputs["h_A"])
    alpha = f32(inputs["alpha"])
    pool = np.asarray(inputs["pool_vectors"], dtype=np.float32)
    U_END = D_B * R
    V_END = U_END + R * D_A
    B_END = V_END + D_B
    Upool = np.ascontiguousarray(pool[:, :U_END])
    Vpool = np.ascontiguousarray(pool[:, U_END:V_END])
    biasE = np.ascontiguousarray(pool[:, V_END:B_END])
    W_base = f32(inputs["W_base"])
    b_base = f32(inputs["b_base"]).reshape(D_B)
    gamma = f32(inputs["gamma"]).reshape(1, 1)
    ln_scale = f32(inputs["ln_scale"]).reshape(D_A)
    ln_bias = f32(inputs["ln_bias"]).reshape(D_A)

    in_maps = []
    for i in range(NC_COUNT):
        sl = slice(i * BS, (i + 1) * BS)
        in_maps.append(
            {
                "h_A": np.ascontiguousarray(h_A[sl]),
                "alpha": np.ascontiguousarray(alpha[sl]),
                "Upool": Upool,
                "Vpool": Vpool,
                "biasE": biasE,
                "W_base": W_base,
                "b_base": b_base,
                "gamma": gamma,
                "ln_scale": ln_scale,
                "ln_bias": ln_bias,
            }
        )
    return in_maps


def run_kernel(trace=False, **inputs):
    from concourse.bass_utils import run_bass_kernel_spmd

    nc = _get_nc()
    in_maps = make_in_maps(**inputs)
    res = run_bass_kernel_spmd(nc, in_maps, core_ids=list(range(NC_COUNT)), trace=trace)
    out = np.concatenate([r["out"] for r in res.results], axis=0)
    return out.astype(np.float32), res


def kernel(**inputs) -> np.ndarray:
    out, _ = run_kernel(trace=False, **inputs)
    return out


# revision 7
# speedup vs baseline: 1.3186x; 1.3186x over previous
"""Bass/Trainium2 kernel for nn_DWAMiddleLayer (low-rank MoE weight-assembly layer).

Math (reference):
    U    = pool[:, :1024].reshape(N, DB, R)      # [512, 256, 4]
    V    = pool[:, 1024:2048].reshape(N, R, DA)  # [512, 4, 256]
    bE   = pool[:, 2048:2304]                    # [512, 256]
    h_t  = h_A @ W_base.T
           + sum_r (alpha * (h_A @ V_r.T)) @ U_r          # never materialize W_assembled
           + alpha @ bE + b_base
    y    = h_A + gamma * h_t ; out = LayerNorm(y) * ln_scale + ln_bias

Distribution: data-parallel over batch B=2048 across 8 cores (BS=256 rows each);
pool/W_base/vectors replicated. h_t is computed in transposed space (feature dim
on partitions, batch on the free dim) so that every matmul contraction dim lands
on partitions naturally; transposes of h_A / alpha / V are done on the PE as
bf16 identity-matmuls. All matmul operands are bf16 (the gamma=1e-2 residual
scaling makes matmul rounding error negligible in the output); the residual +
LayerNorm path uses the untransposed fp32 h_A directly.
"""

import numpy as np

B, N, D_A, D_B, R = 2048, 512, 256, 256, 4
NC_COUNT = 8
BS = B // NC_COUNT  # 256 batch rows per core
P = 128
LN_EPS = 1e-5

_cache = {}


def _build_nc():
    import concourse.mybir as mybir
    from concourse import bacc
    import concourse.tile as tile
    from concourse.masks import make_identity

    fp32 = mybir.dt.float32
    bf16 = mybir.dt.bfloat16

    nc = bacc.Bacc("TRN2", target_bir_lowering=False)

    # ---- DRAM I/O (per-core shard shapes) ----
    d_hA = nc.dram_tensor("h_A", [BS, D_A], fp32, kind="ExternalInput")
    d_alpha = nc.dram_tensor("alpha", [BS, N], fp32, kind="ExternalInput")
    d_U = nc.dram_tensor("Upool", [N, D_B * R], fp32, kind="ExternalInput")
    d_V = nc.dram_tensor("Vpool", [N, R * D_A], fp32, kind="ExternalInput")
    d_bE = nc.dram_tensor("biasE", [N, D_B], fp32, kind="ExternalInput")
    d_Wb = nc.dram_tensor("W_base", [D_B, D_A], fp32, kind="ExternalInput")
    d_bb = nc.dram_tensor("b_base", [D_B], fp32, kind="ExternalInput")
    d_gamma = nc.dram_tensor("gamma", [1, 1], fp32, kind="ExternalInput")
    d_lsc = nc.dram_tensor("ln_scale", [D_A], fp32, kind="ExternalInput")
    d_lbi = nc.dram_tensor("ln_bias", [D_A], fp32, kind="ExternalInput")
    d_out = nc.dram_tensor("out", [BS, D_A], fp32, kind="ExternalOutput")

    with tile.TileContext(nc) as tc:
        with (
            tc.tile_pool(name="persist", bufs=1) as persist,
            tc.tile_pool(name="stage", bufs=2) as stage,
            tc.tile_pool(name="sm", bufs=3) as sm,
            tc.tile_pool(name="pp_tr", bufs=2, space="PSUM") as pp_tr,
            tc.tile_pool(name="pp_t", bufs=2, space="PSUM") as pp_t,
            tc.tile_pool(name="pp_acc", bufs=1, space="PSUM") as pp_acc,
            tc.tile_pool(name="pp_y", bufs=1, space="PSUM") as pp_y,
        ):
            # ---------- small loads ----------
            # h_A is the only operand needed in fp32 (residual path); everything
            # else is loaded as bf16 directly via SWDGE cast-DMAs (the SDMA
            # engines convert inline, HBM read bytes unchanged).
            hA_sb = persist.tile([P, 2, D_A], fp32)  # [p, b_chunk, a]
            nc.sync.dma_start(hA_sb, d_hA[:].rearrange("(o p) a -> p o a", p=P))
            alpha_bf = persist.tile([P, 2, N], bf16)
            nc.gpsimd.dma_start(alpha_bf, d_alpha[:].rearrange("(o p) n -> p o n", p=P))
            Wb_bf = persist.tile([P, 2, D_A], bf16)
            nc.gpsimd.dma_start(Wb_bf, d_Wb[:].rearrange("(o p) a -> p o a", p=P))
            bE_bf = persist.tile([P, 4, D_B], bf16)
            nc.gpsimd.dma_start(bE_bf, d_bE[:].rearrange("(o p) c -> p o c", p=P))

            # broadcast rows / misc constants
            lsc_row = persist.tile([P, D_A], fp32)
            nc.gpsimd.dma_start(lsc_row, d_lsc[:].partition_broadcast(P))
            lbi_row = persist.tile([P, D_A], fp32)
            nc.gpsimd.dma_start(lbi_row, d_lbi[:].partition_broadcast(P))
            gamma_col = persist.tile([P, 1], fp32)
            nc.gpsimd.dma_start(gamma_col, d_gamma[:].to_broadcast([P, 1]))
            bb_row = persist.tile([1, D_B], fp32)  # b_base on one partition
            nc.gpsimd.dma_start(bb_row, d_bb[:].unsqueeze(0))

            eps_col = persist.tile([P, 1], fp32)
            nc.vector.memset(eps_col, LN_EPS)
            ident_b = persist.tile([P, P], bf16)
            make_identity(nc, ident_b)
            ones_row = persist.tile([1, BS], bf16)
            nc.vector.memset(ones_row, 1.0)
            bb_bf = persist.tile([1, D_B], bf16)
            nc.vector.tensor_copy(bb_bf, bb_row)

            # ---------- transposes of small operands (PE identity-matmul, bf16) ----------
            # h_A^T (bf16, matmul use only; the fp32 residual uses hA_sb directly)
            hA_bf = sm.tile([P, 2, D_A], bf16, tag="hAbf")
            nc.vector.tensor_copy(hA_bf, hA_sb)
            hAT_b = persist.tile([P, 2, BS], bf16)  # [p_a, a_chunk, b]
            for ach in range(2):
                ps = pp_tr.tile([P, 512], fp32, tag="tr")
                for bch in range(2):
                    nc.tensor.matmul(
                        ps[:, bch * P : (bch + 1) * P],
                        lhsT=hA_bf[:, bch, ach * P : (ach + 1) * P],
                        rhs=ident_b,
                        start=True,
                        stop=True,
                    )
                nc.any.tensor_copy(hAT_b[:, ach], ps[:, :BS])

            # alpha^T -> bf16 [p_n, n_chunk, b]
            alphaT_b = persist.tile([P, 4, BS], bf16)
            for och in range(4):
                ps = pp_tr.tile([P, 512], fp32, tag="tr")
                for bch in range(2):
                    nc.tensor.matmul(
                        ps[:, bch * P : (bch + 1) * P],
                        lhsT=alpha_bf[:, bch, och * P : (och + 1) * P],
                        rhs=ident_b,
                        start=True,
                        stop=True,
                    )
                nc.any.tensor_copy(alphaT_b[:, och], ps[:, :BS])

            # W_base^T -> bf16 [p_a, a_chunk, c]
            WbT_b = persist.tile([P, 2, D_B], bf16)
            for ach in range(2):
                ps = pp_tr.tile([P, 512], fp32, tag="tr")
                for cch in range(2):
                    nc.tensor.matmul(
                        ps[:, cch * P : (cch + 1) * P],
                        lhsT=Wb_bf[:, cch, ach * P : (ach + 1) * P],
                        rhs=ident_b,
                        start=True,
                        stop=True,
                    )
                nc.any.tensor_copy(WbT_b[:, ach], ps[:, :D_B])

            # ---------- h_t^T accumulator: 2 psum tiles [c_half, b] ----------
            htT = [
                pp_acc.tile([P, BS], fp32, tag=f"acc{ch}", name=f"htT{ch}")
                for ch in range(2)
            ]

            # base-mm: h_t^T += W_base^T-chunks @ h_A^T   (contraction over a)
            for ch in range(2):
                for ach in range(2):
                    nc.tensor.matmul(
                        htT[ch],
                        lhsT=WbT_b[:, ach, ch * P : (ch + 1) * P],
                        rhs=hAT_b[:, ach],
                        start=(ach == 0),
                        stop=False,
                        skip_group_check=True,
                    )
            # b_base: rank-1 update b_base[c] * ones[b]
            for ch in range(2):
                nc.tensor.matmul(
                    htT[ch],
                    lhsT=bb_bf[:, ch * P : (ch + 1) * P],
                    rhs=ones_row,
                    start=False,
                    stop=False,
                    skip_group_check=True,
                )
            # bias-mm: h_t^T += biasE^T-chunks @ alpha^T  (contraction over n)
            for ch in range(2):
                for och in range(4):
                    nc.tensor.matmul(
                        htT[ch],
                        lhsT=bE_bf[:, och, ch * P : (ch + 1) * P],
                        rhs=alphaT_b[:, och],
                        start=False,
                        stop=False,
                        skip_group_check=True,
                    )

            # ---------- main pipeline over expert chunks (o = n//128) ----------
            # V layout per pool row: f = r*256 + a  (r-major)
            # U layout per pool row: f = c*4 + r    (c-major)
            VT_b = persist.tile([P, 2, 2048], bf16)  # [p_a, a_chunk, r*512+o*128+pn]
            U_bfr = persist.tile([P, 4, R, D_B], bf16)  # [p_n, o, r, c]

            for o in range(4):
                V_bf = stage.tile([P, R * D_A], bf16, tag="vbf")
                nc.gpsimd.dma_start(V_bf, d_V[o * P : (o + 1) * P, :])

                # transpose V chunk: blocks (r, a_half) of [128n x 128a]
                for ach in range(2):
                    ps = pp_tr.tile([P, 512], fp32, tag="tr")
                    for r in range(4):
                        nc.tensor.matmul(
                            ps[:, r * P : (r + 1) * P],
                            lhsT=V_bf[:, r * D_A + ach * P : r * D_A + (ach + 1) * P],
                            rhs=ident_b,
                            start=True,
                            stop=True,
                        )
                    # scatter the 4 r-blocks into VT at [r*512 + o*128]
                    dst = VT_b[:, ach].rearrange("p (r q) -> p r q", r=4)[
                        :, :, o * P : (o + 1) * P
                    ]
                    nc.any.tensor_copy(dst, ps[:].rearrange("p (r q) -> p r q", r=4))

                U_bf = stage.tile([P, D_B * R], bf16, tag="ubf")
                nc.gpsimd.dma_start(U_bf, d_U[o * P : (o + 1) * P, :])
                # destride (c r) -> (r c) in bf16 on DVE
                nc.vector.tensor_copy(
                    U_bfr[:, o],
                    U_bf.rearrange("p (c r) -> p r c", r=R),
                )

                for r in range(4):
                    # mm1: t_r^T[n_chunk, b] = V_r @ h_A^T (contract a)
                    t_ps = pp_t.tile([P, BS], fp32, tag="t")
                    for ach in range(2):
                        nc.tensor.matmul(
                            t_ps,
                            lhsT=VT_b[:, ach, r * 512 + o * P : r * 512 + (o + 1) * P],
                            rhs=hAT_b[:, ach],
                            start=(ach == 0),
                            stop=(ach == 1),
                        )
                    # s_r^T = alpha^T * t_r^T  (evict psum -> bf16 sbuf)
                    s_bf = sm.tile([P, BS], bf16, tag="s")
                    nc.vector.tensor_mul(s_bf, t_ps, alphaT_b[:, o])
                    # mm2: h_t^T += U_r^T-chunks @ s_r^T (contract n)
                    for ch in range(2):
                        nc.tensor.matmul(
                            htT[ch],
                            lhsT=U_bfr[:, o, r, ch * P : (ch + 1) * P],
                            rhs=s_bf,
                            start=False,
                            stop=(o == 3 and r == 3),
                            skip_group_check=True,
                        )

            # ---------- epilogue: transpose h_t back, residual + LayerNorm in fp32 ----------
            htT_bf = sm.tile([P, 2, BS], bf16, tag="htTbf")
            for ch in range(2):
                nc.any.tensor_copy(htT_bf[:, ch], htT[ch])

            ht_ps = [
                pp_y.tile([P, D_A], fp32, tag=f"y{b}", name=f"ht_ps{b}")
                for b in range(2)
            ]
            for bch in range(2):
                for jch in range(2):
                    nc.tensor.matmul(
                        ht_ps[bch][:, jch * P : (jch + 1) * P],
                        lhsT=htT_bf[:, jch, bch * P : (bch + 1) * P],
                        rhs=ident_b,
                        start=True,
                        stop=True,
                        skip_group_check=True,
                    )

            out_sb = sm.tile([P, 2, D_A], fp32, tag="out")
            for bch in range(2):
                # y = h_A + gamma * h_t   (fp32 residual)
                y_sb = sm.tile([P, D_A], fp32, tag="y")
                nc.vector.scalar_tensor_tensor(
                    y_sb,
                    in0=ht_ps[bch],
                    scalar=gamma_col,
                    in1=hA_sb[:, bch],
                    op0=mybir.AluOpType.mult,
                    op1=mybir.AluOpType.add,
                )
                stats = sm.tile([P, 6], fp32, tag="st")
                nc.vector.bn_stats(stats, y_sb)
                mv = sm.tile([P, 2], fp32, tag="mv")
                nc.vector.bn_aggr(mv, stats)
                # rstd = 1/sqrt(var + eps)
                rstd = sm.tile([P, 1], fp32, tag="rstd")
                nc.scalar.activation(
                    rstd, mv[:, 1:2], mybir.ActivationFunctionType.Sqrt, bias=eps_col
                )
                nc.vector.reciprocal(rstd, rstd)
                # (y - mu) * rstd
                nc.vector.tensor_scalar(
                    out_sb[:, bch],
                    y_sb,
                    scalar1=mv[:, 0:1],
                    scalar2=rstd,
                    op0=mybir.AluOpType.subtract,
                    op1=mybir.AluOpType.mult,
                )
                # * ln_scale + ln_bias
                nc.vector.tensor_mul(out_sb[:, bch], out_sb[:, bch], lsc_row)
                nc.vector.tensor_add(out_sb[:, bch], out_sb[:, bch], lbi_row)

            nc.sync.dma_start(d_out[:].rearrange("(o p) c -> p o c", p=P), out_sb)

    nc.compile()
    return nc


def _get_nc():
    if "nc" not in _cache:
        _cache["nc"] = _build_nc()
    return _cache["nc"]


def make_in_maps(**inputs):
    """Shard full inputs into 8 per-core input maps."""
    f32 = lambda x: np.ascontiguousarray(np.asarray(x), dtype=np.float32)
    h_A = f32(inputs["h_A"])
    alpha = f32(inputs["alpha"])
    pool = np.asarray(inputs["pool_vectors"], dtype=np.float32)
    U_END = D_B * R
    V_END = U_END + R * D_A
    B_END = V_END + D_B
    Upool = np.ascontiguousarray(pool[:, :U_END])
    Vpool = np.ascontiguousarray(pool[:, U_END:V_END])
    biasE = np.ascontiguousarray(pool[:, V_END:B_END])
    W_base = f32(inputs["W_base"])
    b_base = f32(inputs["b_base"]).reshape(D_B)
    gamma = f32(inputs["gamma"]).reshape(1, 1)
    ln_scale = f32(inputs["ln_scale"]).reshape(D_A)
    ln_bias = f32(inputs["ln_bias"]).reshape(D_A)

    in_maps = []
    for i in range(NC_COUNT):
        sl = slice(i * BS, (i + 1) * BS)
        in_maps.append(
            {
                "h_A": np.ascontiguousarray(h_A[sl]),
                "alpha": np.ascontiguousarray(alpha[sl]),
                "Upool": Upool,
                "Vpool": Vpool,
                "biasE": biasE,
                "W_base": W_base,
                "b_base": b_base,
                "gamma": gamma,
                "ln_scale": ln_scale,
                "ln_bias": ln_bias,
            }
        )
    return in_maps


def run_kernel(trace=False, **inputs):
    from concourse.bass_utils import run_bass_kernel_spmd

    nc = _get_nc()
    in_maps = make_in_maps(**inputs)
    res = run_bass_kernel_spmd(nc, in_maps, core_ids=list(range(NC_COUNT)), trace=trace)
    out = np.concatenate([r["out"] for r in res.results], axis=0)
    return out.astype(np.float32), res


def kernel(**inputs) -> np.ndarray:
    out, _ = run_kernel(trace=False, **inputs)
    return out


# revision 8
# speedup vs baseline: 1.4124x; 1.0711x over previous
"""Bass/Trainium2 kernel for nn_DWAMiddleLayer (low-rank MoE weight-assembly layer).

Math (reference):
    U    = pool[:, :1024].reshape(N, DB, R)      # [512, 256, 4]
    V    = pool[:, 1024:2048].reshape(N, R, DA)  # [512, 4, 256]
    bE   = pool[:, 2048:2304]                    # [512, 256]
    h_t  = h_A @ W_base.T
           + sum_r (alpha * (h_A @ V_r.T)) @ U_r          # never materialize W_assembled
           + alpha @ bE + b_base
    y    = h_A + gamma * h_t ; out = LayerNorm(y) * ln_scale + ln_bias

Distribution: data-parallel over batch B=2048 across 8 cores (BS=256 rows each);
pool/W_base/vectors replicated. h_t is computed in transposed space (feature dim
on partitions, batch on the free dim) so that every matmul contraction dim lands
on partitions naturally; transposes of h_A / alpha / V are done on the PE as
bf16 identity-matmuls. All matmul operands are bf16 (loaded via SWDGE cast-DMAs;
the gamma=1e-2 residual scaling makes matmul rounding error negligible in the
output); the residual + LayerNorm path uses the untransposed fp32 h_A directly.
"""

import numpy as np

B, N, D_A, D_B, R = 2048, 512, 256, 256, 4
NC_COUNT = 8
BS = B // NC_COUNT  # 256 batch rows per core
P = 128
LN_EPS = 1e-5
POOL_W = D_B * R + R * D_A + D_B  # 2304 used columns of pool_vectors
U_OFF, V_OFF, BE_OFF = 0, D_B * R, D_B * R + R * D_A

_cache = {}


def _build_nc():
    import concourse.mybir as mybir
    import concourse.tile as tile
    from concourse import bacc
    from concourse.masks import make_identity

    fp32 = mybir.dt.float32
    bf16 = mybir.dt.bfloat16

    nc = bacc.Bacc("TRN2", target_bir_lowering=False)

    # ---- DRAM I/O (per-core shard shapes) ----
    d_hA = nc.dram_tensor("h_A", [BS, D_A], fp32, kind="ExternalInput")
    d_alpha = nc.dram_tensor("alpha", [BS, N], fp32, kind="ExternalInput")
    d_UV = nc.dram_tensor("UVpool", [N, POOL_W], fp32, kind="ExternalInput")
    d_Wb = nc.dram_tensor("W_base", [D_B, D_A], fp32, kind="ExternalInput")
    d_bb = nc.dram_tensor("b_base", [D_B], fp32, kind="ExternalInput")
    d_gamma = nc.dram_tensor("gamma", [1, 1], fp32, kind="ExternalInput")
    d_lsc = nc.dram_tensor("ln_scale", [D_A], fp32, kind="ExternalInput")
    d_lbi = nc.dram_tensor("ln_bias", [D_A], fp32, kind="ExternalInput")
    d_out = nc.dram_tensor("out", [BS, D_A], fp32, kind="ExternalOutput")

    with tile.TileContext(nc) as tc:
        with (
            tc.tile_pool(name="persist", bufs=1) as persist,
            tc.tile_pool(name="stage", bufs=3) as stage,
            tc.tile_pool(name="sm", bufs=3) as sm,
            tc.tile_pool(name="pp_tr", bufs=3, space="PSUM") as pp_tr,
            tc.tile_pool(name="pp_t", bufs=2, space="PSUM") as pp_t,
            tc.tile_pool(name="pp_acc", bufs=1, space="PSUM") as pp_acc,
        ):
            # ---------- constants first: emitted before any SWDGE DMA so the
            # gpsimd queue produces the identity immediately ----------
            ident_b = persist.tile([P, P], bf16)
            make_identity(nc, ident_b)
            eps_col = persist.tile([P, 1], fp32)
            nc.vector.memset(eps_col, LN_EPS)
            ones_row = persist.tile([1, BS], bf16)
            nc.vector.memset(ones_row, 1.0)
            # warm the ACT Sqrt table early so the LN tail doesn't pay the
            # 1.3us ACT_TABLE_LOAD on the critical path
            warm = sm.tile([P, 1], fp32, tag="warm")
            nc.scalar.activation(
                warm, eps_col, mybir.ActivationFunctionType.Sqrt, bias=eps_col
            )

            # ---------- loads ----------
            # h_A fp32 on HWDGE (only fp32 consumer: residual + its own bf16 cast)
            hA_sb = persist.tile([P, 2, D_A], fp32)  # [p, b_chunk, a]
            nc.sync.dma_start(hA_sb, d_hA[:].rearrange("(o p) a -> p o a", p=P))
            hA_bf = sm.tile([P, 2, D_A], bf16, tag="hAbf")
            nc.vector.tensor_copy(hA_bf, hA_sb)

            # pool chunks via SWDGE cast-DMA (fp32 HBM read -> bf16 SBUF write);
            # chunk 0 goes out before the small loads
            UVc = [
                stage.tile([P, POOL_W], bf16, tag="uvc", name=f"UVc{o}")
                for o in range(4)
            ]
            nc.gpsimd.dma_start(UVc[0], d_UV[0 * P : 1 * P, :])
            alpha_bf = persist.tile([P, 2, N], bf16)
            nc.gpsimd.dma_start(alpha_bf, d_alpha[:].rearrange("(o p) n -> p o n", p=P))
            nc.gpsimd.dma_start(UVc[1], d_UV[1 * P : 2 * P, :])
            Wb_bf = persist.tile([P, 2, D_A], bf16)
            nc.gpsimd.dma_start(Wb_bf, d_Wb[:].rearrange("(o p) a -> p o a", p=P))
            bb_row = persist.tile([1, D_B], fp32)
            nc.gpsimd.dma_start(bb_row, d_bb[:].unsqueeze(0))
            bb_bf = persist.tile([1, D_B], bf16)
            nc.vector.tensor_copy(bb_bf, bb_row)
            nc.gpsimd.dma_start(UVc[2], d_UV[2 * P : 3 * P, :])
            # epilogue-only constants late
            lsc_row = persist.tile([P, D_A], fp32)
            nc.gpsimd.dma_start(lsc_row, d_lsc[:].partition_broadcast(P))
            lbi_row = persist.tile([P, D_A], fp32)
            nc.gpsimd.dma_start(lbi_row, d_lbi[:].partition_broadcast(P))
            gamma_col = persist.tile([P, 1], fp32)
            nc.gpsimd.dma_start(gamma_col, d_gamma[:].to_broadcast([P, 1]))
            nc.gpsimd.dma_start(UVc[3], d_UV[3 * P : 4 * P, :])

            # ---------- transposes of small operands (PE identity-matmul, bf16) ----------
            hAT_b = persist.tile([P, 2, BS], bf16)  # [p_a, a_chunk, b]
            for ach in range(2):
                ps = pp_tr.tile([P, 512], fp32, tag="tr")
                for bch in range(2):
                    nc.tensor.matmul(
                        ps[:, bch * P : (bch + 1) * P],
                        lhsT=hA_bf[:, bch, ach * P : (ach + 1) * P],
                        rhs=ident_b,
                        start=True,
                        stop=True,
                    )
                nc.any.tensor_copy(hAT_b[:, ach], ps[:, :BS])

            # alpha^T -> bf16 [p_n, n_chunk, b]
            alphaT_b = persist.tile([P, 4, BS], bf16)
            for och in range(4):
                ps = pp_tr.tile([P, 512], fp32, tag="tr")
                for bch in range(2):
                    nc.tensor.matmul(
                        ps[:, bch * P : (bch + 1) * P],
                        lhsT=alpha_bf[:, bch, och * P : (och + 1) * P],
                        rhs=ident_b,
                        start=True,
                        stop=True,
                    )
                nc.any.tensor_copy(alphaT_b[:, och], ps[:, :BS])

            # W_base^T -> bf16 [p_a, a_chunk, c]
            WbT_b = persist.tile([P, 2, D_B], bf16)
            for ach in range(2):
                ps = pp_tr.tile([P, 512], fp32, tag="tr")
                for cch in range(2):
                    nc.tensor.matmul(
                        ps[:, cch * P : (cch + 1) * P],
                        lhsT=Wb_bf[:, cch, ach * P : (ach + 1) * P],
                        rhs=ident_b,
                        start=True,
                        stop=True,
                    )
                nc.any.tensor_copy(WbT_b[:, ach], ps[:, :D_B])

            # ---------- h_t^T accumulator: 2 psum tiles [c_half, b] ----------
            htT = [
                pp_acc.tile([P, BS], fp32, tag=f"acc{ch}", name=f"htT{ch}")
                for ch in range(2)
            ]
            started = [False, False]

            def acc_mm(ch, lhsT, rhs, last=False):
                nc.tensor.matmul(
                    htT[ch],
                    lhsT=lhsT,
                    rhs=rhs,
                    start=(not started[ch]),
                    stop=last,
                    skip_group_check=True,
                )
                started[ch] = True

            # ---------- main pipeline over expert chunks (o = n//128) ----------
            # V layout per pool row: f = V_OFF + r*256 + a  (r-major)
            # U layout per pool row: f = c*4 + r            (c-major)
            VT_b = persist.tile([P, 2, 2048], bf16)  # [p_a, a_chunk, r*512+o*128+pn]
            U_bfr = persist.tile([P, 4, R, D_B], bf16)  # [p_n, o, r, c]

            for o in range(4):
                V_bf = UVc[o][:, V_OFF : V_OFF + R * D_A]
                # transpose V chunk: blocks (r, a_half) of [128n x 128a]
                for ach in range(2):
                    ps = pp_tr.tile([P, 512], fp32, tag="tr")
                    for r in range(4):
                        nc.tensor.matmul(
                            ps[:, r * P : (r + 1) * P],
                            lhsT=V_bf[:, r * D_A + ach * P : r * D_A + (ach + 1) * P],
                            rhs=ident_b,
                            start=True,
                            stop=True,
                        )
                    # scatter the 4 r-blocks into VT at [r*512 + o*128]
                    dst = VT_b[:, ach].rearrange("p (r q) -> p r q", r=4)[
                        :, :, o * P : (o + 1) * P
                    ]
                    nc.any.tensor_copy(dst, ps[:].rearrange("p (r q) -> p r q", r=4))

                # destride U chunk (c r) -> (r c) in bf16 on DVE
                nc.vector.tensor_copy(
                    U_bfr[:, o],
                    UVc[o][:, U_OFF : U_OFF + D_B * R].rearrange(
                        "p (c r) -> p r c", r=R
                    ),
                )

                for r in range(4):
                    # mm1: t_r^T[n_chunk, b] = V_r @ h_A^T (contract a)
                    t_ps = pp_t.tile([P, BS], fp32, tag="t")
                    for ach in range(2):
                        nc.tensor.matmul(
                            t_ps,
                            lhsT=VT_b[:, ach, r * 512 + o * P : r * 512 + (o + 1) * P],
                            rhs=hAT_b[:, ach],
                            start=(ach == 0),
                            stop=(ach == 1),
                        )
                    # s_r^T = alpha^T * t_r^T  (evict psum -> bf16 sbuf)
                    s_bf = sm.tile([P, BS], bf16, tag="s")
                    nc.vector.tensor_mul(s_bf, t_ps, alphaT_b[:, o])
                    # mm2: h_t^T += U_r^T-chunks @ s_r^T (contract n)
                    for ch in range(2):
                        acc_mm(ch, U_bfr[:, o, r, ch * P : (ch + 1) * P], s_bf)

                # bias-mm for this chunk: h_t^T += biasE^T @ alpha^T (contract n)
                bE_o = UVc[o][:, BE_OFF : BE_OFF + D_B]
                for ch in range(2):
                    acc_mm(ch, bE_o[:, ch * P : (ch + 1) * P], alphaT_b[:, o], last=(o == 3))

                if o == 0:
                    # base-mm + b_base rank-1, folded in early (no DMA deps left)
                    for ch in range(2):
                        for ach in range(2):
                            acc_mm(
                                ch, WbT_b[:, ach, ch * P : (ch + 1) * P], hAT_b[:, ach]
                            )
                        acc_mm(ch, bb_bf[:, ch * P : (ch + 1) * P], ones_row)

            # ---------- epilogue: transpose h_t back, residual + LayerNorm in fp32 ----------
            htT_bf = sm.tile([P, 2, BS], bf16, tag="htTbf")
            for ch in range(2):
                nc.any.tensor_copy(htT_bf[:, ch], htT[ch])

            ht_ps = pp_tr.tile([P, 512], fp32, tag="tr", name="ht_ps")
            for bch in range(2):
                for jch in range(2):
                    nc.tensor.matmul(
                        ht_ps[:, bch * 256 + jch * P : bch * 256 + (jch + 1) * P],
                        lhsT=htT_bf[:, jch, bch * P : (bch + 1) * P],
                        rhs=ident_b,
                        start=True,
                        stop=True,
                        skip_group_check=True,
                    )

            out_sb = sm.tile([P, 2, D_A], fp32, tag="out")
            for bch in range(2):
                # y = h_A + gamma * h_t   (fp32 residual)
                y_sb = sm.tile([P, D_A], fp32, tag="y")
                nc.vector.scalar_tensor_tensor(
                    y_sb,
                    in0=ht_ps[:, bch * 256 : bch * 256 + D_A],
                    scalar=gamma_col,
                    in1=hA_sb[:, bch],
                    op0=mybir.AluOpType.mult,
                    op1=mybir.AluOpType.add,
                )
                stats = sm.tile([P, 6], fp32, tag="st")
                nc.vector.bn_stats(stats, y_sb)
                mv = sm.tile([P, 2], fp32, tag="mv")
                nc.vector.bn_aggr(mv, stats)
                # rstd = 1/sqrt(var + eps)
                rstd = sm.tile([P, 1], fp32, tag="rstd")
                nc.scalar.activation(
                    rstd, mv[:, 1:2], mybir.ActivationFunctionType.Sqrt, bias=eps_col
                )
                nc.vector.reciprocal(rstd, rstd)
                # (y - mu) * rstd
                nc.vector.tensor_scalar(
                    out_sb[:, bch],
                    y_sb,
                    scalar1=mv[:, 0:1],
                    scalar2=rstd,
                    op0=mybir.AluOpType.subtract,
                    op1=mybir.AluOpType.mult,
                )
                # * ln_scale + ln_bias
                nc.vector.tensor_mul(out_sb[:, bch], out_sb[:, bch], lsc_row)
                nc.vector.tensor_add(out_sb[:, bch], out_sb[:, bch], lbi_row)

            nc.sync.dma_start(d_out[:].rearrange("(o p) c -> p o c", p=P), out_sb)

    nc.compile()
    return nc


def _get_nc():
    if "nc" not in _cache:
        _cache["nc"] = _build_nc()
    return _cache["nc"]


def make_in_maps(**inputs):
    """Shard full inputs into 8 per-core input maps."""
    f32 = lambda x: np.ascontiguousarray(np.asarray(x), dtype=np.float32)
    h_A = f32(inputs["h_A"])
    alpha = f32(inputs["alpha"])
    pool = np.asarray(inputs["pool_vectors"], dtype=np.float32)
    UVpool = np.ascontiguousarray(pool[:, :POOL_W])
    W_base = f32(inputs["W_base"])
    b_base = f32(inputs["b_base"]).reshape(D_B)
    gamma = f32(inputs["gamma"]).reshape(1, 1)
    ln_scale = f32(inputs["ln_scale"]).reshape(D_A)
    ln_bias = f32(inputs["ln_bias"]).reshape(D_A)

    in_maps = []
    for i in range(NC_COUNT):
        sl = slice(i * BS, (i + 1) * BS)
        in_maps.append(
            {
                "h_A": np.ascontiguousarray(h_A[sl]),
                "alpha": np.ascontiguousarray(alpha[sl]),
                "UVpool": UVpool,
                "W_base": W_base,
                "b_base": b_base,
                "gamma": gamma,
                "ln_scale": ln_scale,
                "ln_bias": ln_bias,
            }
        )
    return in_maps


def run_kernel(trace=False, **inputs):
    from concourse.bass_utils import run_bass_kernel_spmd

    nc = _get_nc()
    in_maps = make_in_maps(**inputs)
    res = run_bass_kernel_spmd(nc, in_maps, core_ids=list(range(NC_COUNT)), trace=trace)
    out = np.concatenate([r["out"] for r in res.results], axis=0)
    return out.astype(np.float32), res


def kernel(**inputs) -> np.ndarray:
    out, _ = run_kernel(trace=False, **inputs)
    return out
